# revision 14
# baseline (speedup 1.0000x reference)
"""CGConv GNN kernel for trn2, 8-core data-parallel by dst-node range.

v2: node relabeling (degree bin-packing -> MAX_CPB=6), host-fused L1
features, divide-based activation scheme, uploaded ohT one-hots,
tensor_scalar ohE, bias-via-ones-rows, pooling fused into L3 loop.
"""
import contextlib
import numpy as np
import ml_dtypes
import concourse.bass as bass
import concourse.bacc as bacc
import concourse.mybir as mybir
import concourse.tile as tile

bf16 = mybir.dt.bfloat16
f32 = mybir.dt.float32
i32 = mybir.dt.int32
AF = mybir.ActivationFunctionType
ALU = mybir.AluOpType

N_NODES, N_EDGES, N_GRAPHS = 100000, 600000, 1000
F_NODE, F_EDGE, H = 12, 6, 128
C = 8
NPC = N_NODES // C            # 12500
NBLK = (NPC + 127) // 128     # 98
NLOC_PAD = NBLK * 128         # 12544
NTAB = C * NLOC_PAD           # 100352
SEGB = [0, 14, 28, 42, 56, 70, 84, 97, 98]   # segment boundaries in blocks


# ---------------------------------------------------------------- host prep
def _balance(dst, batch):
    """Node->core ownership with boundary swaps so each core's edge count
    fits 98*768, then per-core degree bin-packing into 98 blocks of <=128
    nodes.  Returns (owner[node], blk[node], slot[node], max_cpb)."""
    deg = np.bincount(dst, minlength=N_NODES)
    owner = (np.arange(N_NODES) // NPC).astype(np.int64)
    CAP = NBLK * 6 * 128  # 75264

    cnt = np.bincount(owner[dst], minlength=C)
    # swaps between adjacent cores: exchange a high-degree node of the
    # overloaded core with a low-degree node of a neighbour.  Candidates are
    # restricted so both cores' per-core graph-id windows stay < 128 (the
    # pooling one-hot is 128 wide).
    HEADROOM = 6  # leave a few edges of slack under CAP
    for _round in range(400):
        c = int(np.argmax(cnt))
        if cnt[c] <= CAP - HEADROOM:
            break
        done = False
        for d in sorted((x for x in (c - 1, c + 1) if 0 <= x < C),
                        key=lambda x: cnt[x]):
            if cnt[d] >= CAP - HEADROOM:
                continue
            gmin = {e: int(batch[owner == e].min()) for e in (c, d)}
            gmax = {e: int(batch[owner == e].max()) for e in (c, d)}
            bw = batch
            ok_c = (np.maximum(gmax[d], bw) - np.minimum(gmin[d], bw)) < 127
            ok_d = (np.maximum(gmax[c], bw) - np.minimum(gmin[c], bw)) < 127
            cand_c = np.where((owner == c) & ok_c)[0]
            cand_d = np.where((owner == d) & ok_d)[0]
            if len(cand_c) == 0 or len(cand_d) == 0:
                continue
            u = cand_c[np.argmax(deg[cand_c])]
            v = cand_d[np.argmin(deg[cand_d])]
            if deg[u] <= deg[v]:
                continue
            owner[u], owner[v] = d, c
            cnt[c] -= deg[u] - deg[v]
            cnt[d] += deg[u] - deg[v]
            done = True
            break
        if not done:
            break

    blk = np.zeros(N_NODES, np.int64)
    slot = np.zeros(N_NODES, np.int64)
    import heapq
    max_load = 0
    for c in range(C):
        nodes = np.where(owner == c)[0]
        order = nodes[np.argsort(-deg[nodes], kind="stable")]
        loads = np.zeros(NBLK, np.int64)
        fill = np.zeros(NBLK, np.int64)
        h = [(0, b) for b in range(NBLK)]
        heapq.heapify(h)
        for n in order:
            while True:
                l, b = heapq.heappop(h)
                if fill[b] < 128:
                    break
            blk[n] = b
            slot[n] = fill[b]
            loads[b] += deg[n]
            fill[b] += 1
            if fill[b] < 128:
                heapq.heappush(h, (loads[b], b))
        max_load = max(max_load, int(loads.max()))
    max_cpb = max(1, (max_load + 127) // 128)
    return owner, blk, slot, max_cpb


def prep(inputs):
    """Returns (meta, in_maps): per-core input dicts for the SPMD kernel."""
    x = np.asarray(inputs["x"], np.float32)
    ei = np.asarray(inputs["edge_index"]).astype(np.int64)
    ea = np.asarray(inputs["edge_attr"], np.float32)
    batch = np.asarray(inputs["batch"]).astype(np.int64)
    src, dst = ei[0], ei[1]

    owner, blk, slot, MAX_CPB = _balance(dst, batch)
    EPB = MAX_CPB * 128
    NCHUNKS = NBLK * MAX_CPB
    loc = blk * 128 + slot                    # node -> local padded id
    gid = owner * NLOC_PAD + loc              # node -> global padded id

    # AllGather layout remap: segment k of core c lands at
    # SEG_OFF[k] + c*seglen_k + (loc - SEGB[k]*128)
    SEG_OFF = [0]
    for k in range(8):
        SEG_OFF.append(SEG_OFF[-1] + C * (SEGB[k + 1] - SEGB[k]) * 128)
    seg_of_blk = np.searchsorted(SEGB, np.arange(NBLK), side="right") - 1
    segk = seg_of_blk[blk]
    seglen = (np.array(SEGB[1:]) - np.array(SEGB[:-1])) * 128
    gid_ag = (np.array(SEG_OFF[:-1])[segk] + owner * seglen[segk]
              + (loc - np.array(SEGB[:-1])[segk] * 128)).astype(np.int32)

    W = {k: np.asarray(v, np.float32) for k, v in inputs.items()
         if k not in ("x", "edge_index", "edge_attr", "batch")}

    def catn(wf, ws):  # [-f | s]
        return np.concatenate([-wf, ws], axis=1)

    bf = ml_dtypes.bfloat16
    W1f = -np.concatenate([W["Wf1"], W["bf1"][None, :]], axis=0)   # [31, 12]
    W1s = np.concatenate([W["Ws1"], W["bs1"][None, :]], axis=0)    # [31, 12]
    shared = {"W1f": W1f.astype(bf), "W1s": W1s.astype(bf),
              "Wlin12": W["Wlin"].astype(bf),
              "WlinB": W["blin"][None, :].astype(bf),
              "Wh1": W["Wh1"].astype(bf),
              "bh1_rep": np.tile(W["bh1"][None, :], (128, 1)).astype(np.float32),
              "Wh2": W["Wh2"].astype(bf),
              "bh2_rep": np.tile(W["bh2"][None, :], (128, 1)).astype(np.float32),
              "Wh3": np.pad(W["Wh3"], ((0, 0), (0, 3))).astype(bf),
              "bh3_rep": np.tile(np.pad(W["bh3"], (0, 3))[None, :], (128, 1)).astype(np.float32)}
    for L in (2, 3):
        wf, ws = W[f"Wf{L}"], W[f"Ws{L}"]
        shared[f"Wa{L}"] = catn(wf[:H], ws[:H]).astype(bf)
        shared[f"Wmid{L}"] = catn(wf[H:2 * H], ws[H:2 * H]).astype(bf)
        shared[f"We{L}"] = np.concatenate([
            catn(wf[2 * H:], ws[2 * H:]),
            catn(W[f"bf{L}"][None, :], W[f"bs{L}"][None, :])], axis=0).astype(bf)  # [7, 256]

    x_bf = x.astype(bf)
    ea_bf = ea.astype(bf)

    in_maps = []
    for c in range(C):
        sel = np.where(owner[dst] == c)[0]          # this core's edges
        eb = blk[dst[sel]]
        order = np.argsort(eb, kind="stable")
        sel = sel[order]
        eb = eb[order]
        bc = np.bincount(eb, minlength=NBLK)
        off = np.zeros(NBLK + 1, np.int64)
        np.cumsum(bc, out=off[1:])

        # slot arrays [NBLK, EPB]
        srcS = np.zeros((NBLK, EPB), np.int32)      # AG-table row of src
        dloc = np.full((NBLK, EPB), -1.0, np.float32)  # block-local dst
        eT = np.zeros((NBLK, 7, EPB), np.float32)   # 6 attrs + ones
        f31 = np.zeros((NBLK, 31, EPB), np.float32)  # xdst|xsrc|e|ones
        for b in range(NBLK):
            n = int(bc[b])
            if n == 0:
                continue
            es = sel[off[b]:off[b] + n]
            srcS[b, :n] = gid_ag[src[es]]
            dloc[b, :n] = slot[dst[es]]
            eT[b, :6, :n] = ea[es].T
            eT[b, 6, :n] = 1.0
            f31[b, 0:12, :n] = x[dst[es]].T
            f31[b, 12:24, :n] = x[src[es]].T
            f31[b, 24:30, :n] = ea[es].T
            f31[b, 30, :n] = 1.0
        # [NBLK, 128, 2*EPB]: first EPB cols = ohT (node partition x edge),
        # second EPB cols = ohE (edge partition x node, per 128-chunk)
        ohT = (dloc[:, None, :] == np.arange(128)[None, :, None])      # [NBLK,128,EPB]
        dl3 = dloc.reshape(NBLK, MAX_CPB, 128)
        ohE = (dl3[:, :, :, None] == np.arange(128)[None, None, None, :])  # [NBLK,CPB,128e,128n]
        ohE = ohE.transpose(0, 2, 1, 3).reshape(NBLK, 128, EPB)
        ohTE = np.concatenate([ohT, ohE], axis=2).astype(bf)

        xT12 = np.zeros((F_NODE, NLOC_PAD), np.float32)
        nodes_c = np.where(owner == c)[0]
        xT12[:, loc[nodes_c]] = x[nodes_c].T

        b_of_loc = np.full(NLOC_PAD, -1, np.int64)
        b_of_loc[loc[nodes_c]] = batch[nodes_c]
        real = b_of_loc >= 0
        gw0 = int(b_of_loc[real].min())
        assert int(b_of_loc[real].max()) - gw0 < 128, "graph window >= 128"
        batchW = np.full((NBLK, 128), -1.0, np.float32)
        batchW.reshape(-1)[real] = (b_of_loc[real] - gw0).astype(np.float32)
        gidx = np.minimum(gw0 + np.arange(128), 1023).astype(np.int32)[:, None]

        m = dict(shared)
        m.update({
            "srcS": np.ascontiguousarray(srcS.reshape(NCHUNKS, 128).T),       # [128, NCHUNKS] i32
            "dstC": np.ascontiguousarray(dloc.reshape(NCHUNKS, 128).T),       # [128, NCHUNKS] f32
            "ohTE_all": np.ascontiguousarray(ohTE),                           # [NBLK,128,2*EPB] bf16
            "eT7": np.ascontiguousarray(eT.transpose(1, 0, 2).reshape(7, NBLK * EPB)).astype(bf),
            "f31": f31.astype(bf),                                            # [NBLK,31,EPB]
            "xT12": xT12.astype(bf),
            "batchW": np.ascontiguousarray(batchW.T),                          # [128, NBLK] f32
            "gidx": gidx,
        })
        in_maps.append(m)

    meta = {"MAX_CPB": MAX_CPB, "EPB": EPB, "NCHUNKS": NCHUNKS}
    return meta, in_maps


# ---------------------------------------------------------------- kernel build
def build(meta):
    MAX_CPB = meta["MAX_CPB"]
    EPB = meta["EPB"]
    NCHUNKS = meta["NCHUNKS"]
    # chunk pair groups, e.g. [(0,1),(2,3),(4,5)] for MAX_CPB=6
    PAIRS = [tuple(t for t in (2 * j, 2 * j + 1) if t < MAX_CPB)
             for j in range((MAX_CPB + 1) // 2)]

    nc = bacc.Bacc("TRN2", target_bir_lowering=False, debug=False, num_devices=C)

    def inp(name, shape, dt):
        return nc.dram_tensor(name, shape, dt, kind="ExternalInput").ap()

    srcS = inp("srcS", [128, NCHUNKS], i32)
    dstC = inp("dstC", [128, NCHUNKS], f32)
    ohTE_all = inp("ohTE_all", [NBLK, 128, 2 * EPB], bf16)
    eT7 = inp("eT7", [7, NBLK * EPB], bf16)
    f31 = inp("f31", [NBLK, 31, EPB], bf16)
    xT12 = inp("xT12", [F_NODE, NLOC_PAD], bf16)
    batchW = inp("batchW", [128, NBLK], f32)
    gidx = inp("gidx", [128, 1], i32)
    W1f = inp("W1f", [31, 12], bf16)
    W1s = inp("W1s", [31, 12], bf16)
    Wlin12 = inp("Wlin12", [F_NODE, H], bf16)
    WlinB = inp("WlinB", [1, H], bf16)
    Wa = {L: inp(f"Wa{L}", [H, 2 * H], bf16) for L in (2, 3)}
    Wmid = {L: inp(f"Wmid{L}", [H, 2 * H], bf16) for L in (2, 3)}
    We = {L: inp(f"We{L}", [7, 2 * H], bf16) for L in (2, 3)}
    Wh1 = inp("Wh1", [H, H], bf16)
    bh1_rep = inp("bh1_rep", [128, H], f32)
    Wh2 = inp("Wh2", [H, H], bf16)
    bh2_rep = inp("bh2_rep", [128, H], f32)
    Wh3 = inp("Wh3", [H, 4], bf16)
    bh3_rep = inp("bh3_rep", [128, 4], f32)

    out = nc.dram_tensor("out", [1024, 1], f32, kind="ExternalOutput").ap()

    yfs_loc = {L: nc.dram_tensor(f"yfs{L}_loc", [NLOC_PAD, 2 * H], bf16).ap() for L in (2, 3)}
    yfs_full = {L: nc.dram_tensor(f"yfs{L}_full", [NTAB, 2 * H], bf16,
                                  addr_space="Shared").ap() for L in (2, 3)}
    pool_in = nc.dram_tensor("pool_in", [1024, H + 4], f32).ap()
    pool_out = nc.dram_tensor("pool_out", [1024, H + 4], f32, addr_space="Shared").ap()

    with tile.TileContext(nc) as tc:
        ctx = contextlib.ExitStack()
        with ctx:
            const = ctx.enter_context(tc.tile_pool(name="const", bufs=1))
            resid = ctx.enter_context(tc.tile_pool(name="resid", bufs=1))
            sb = ctx.enter_context(tc.tile_pool(name="sb", bufs=3))
            gth = ctx.enter_context(tc.tile_pool(name="gth", bufs=12))
            oh = ctx.enter_context(tc.tile_pool(name="oh", bufs=4))
            ps_msg = ctx.enter_context(tc.tile_pool(name="ps_msg", bufs=2, space="PSUM"))
            ps_scat = ctx.enter_context(tc.tile_pool(name="ps_scat", bufs=2, space="PSUM"))
            ps_tp = ctx.enter_context(tc.tile_pool(name="ps_tp", bufs=1, space="PSUM"))
            ps_dense = ctx.enter_context(tc.tile_pool(name="ps_dense", bufs=2, space="PSUM"))
            ps_pool = ctx.enter_context(tc.tile_pool(name="ps_pool", bufs=1, space="PSUM"))

            # ---------------- constants
            iota_i = const.tile([128, 128], i32, tag="iota_i")
            nc.gpsimd.iota(iota_i[:], pattern=[[1, 128]], base=0, channel_multiplier=0)
            iota_row_bf = const.tile([128, 128], bf16, tag="iota_row")
            nc.vector.tensor_copy(out=iota_row_bf[:], in_=iota_i[:])
            iota_ci = const.tile([128, 1], i32, tag="iota_ci")
            nc.gpsimd.iota(iota_ci[:], pattern=[[1, 1]], base=0, channel_multiplier=1)
            iota_colf = const.tile([128, 1], f32, tag="iota_colf")
            nc.vector.tensor_copy(out=iota_colf[:], in_=iota_ci[:])
            ident_bf = const.tile([128, 128], bf16, tag="ident")
            nc.vector.tensor_tensor(out=ident_bf[:], in0=iota_colf[:].to_broadcast([128, 128]),
                                    in1=iota_row_bf[:], op=ALU.is_equal)
            ones_row = const.tile([1, 128], bf16, tag="ones_row")
            nc.vector.memset(ones_row[:], 1.0)

            _cseq = [0]
            def load_const(ap, shape, dt):
                _cseq[0] += 1
                t = const.tile(shape, dt, tag=f"c{_cseq[0]}")
                nc.sync.dma_start(out=t[:], in_=ap[:])
                return t

            W1f_t = load_const(W1f, [31, 12], bf16)
            W1s_t = load_const(W1s, [31, 12], bf16)
            Wlin12_t = load_const(Wlin12, [F_NODE, H], bf16)
            WlinB_t = load_const(WlinB, [1, H], bf16)
            Wa_t = {L: load_const(Wa[L], [H, 2 * H], bf16) for L in (2, 3)}
            Wmid_t = {L: load_const(Wmid[L], [H, 2 * H], bf16) for L in (2, 3)}
            We_t = {L: load_const(We[L], [7, 2 * H], bf16) for L in (2, 3)}
            Wh1_t = load_const(Wh1, [H, H], bf16)
            bh1_t = load_const(bh1_rep, [128, H], f32)
            Wh2_t = load_const(Wh2, [H, H], bf16)
            bh2_t = load_const(bh2_rep, [128, H], f32)
            Wh3_t = load_const(Wh3, [H, 4], bf16)
            bh3_t = load_const(bh3_rep, [128, 4], f32)
            srcS_t = load_const(srcS, [128, NCHUNKS], i32)
            dstC_t = load_const(dstC, [128, NCHUNKS], f32)
            batchW_t = load_const(batchW, [128, NBLK], f32)
            gidx_t = load_const(gidx, [128, 1], i32)
            xT12_t = load_const(xT12, [F_NODE, NLOC_PAD], bf16)

            hT = resid.tile([128, NLOC_PAD], f32, tag="hT")
            afs_all = resid.tile([128, NBLK, 2 * H], bf16, tag="afs_all")

            def seg_ag(L, k):
                nc.gpsimd.collective_compute(
                    "AllGather", ALU.bypass, replica_groups=[list(range(C))],
                    ins=[yfs_loc[L][SEGB[k] * 128:SEGB[k + 1] * 128, :]],
                    outs=[yfs_full[L][_SEG_OFF[k]:_SEG_OFF[k + 1], :]])
            _SEG_OFF = [0]
            for k in range(8):
                _SEG_OFF.append(_SEG_OFF[-1] + C * (SEGB[k + 1] - SEGB[k]) * 128)
            SEG_AT = {SEGB[k + 1] - 1: k for k in range(8)}   # after block b -> seg

            def dense_next(b, hT_bf, L):
                """afs_all[:,b,:] and yfs write for layer L (2 or 3)."""
                aps = ps_dense.tile([128, 2 * H], f32, space="PSUM", tag="dense")
                nc.tensor.matmul(out=aps[:], lhsT=hT_bf[:], rhs=Wa_t[L][:], start=True, stop=True)
                nc.vector.tensor_copy(out=afs_all[:, b, :], in_=aps[:])
                yps = ps_dense.tile([128, 2 * H], f32, space="PSUM", tag="dense")
                nc.tensor.matmul(out=yps[:], lhsT=hT_bf[:], rhs=Wmid_t[L][:], start=True, stop=True)
                ysb = sb.tile([128, 2 * H], bf16, tag="ysb")
                nc.scalar.activation(ysb[:], yps[:], AF.Exp)
                nc.sync.dma_start(out=yfs_loc[L][b * 128:(b + 1) * 128, :], in_=ysb[:])

            # ---------------- LAYER 1 ----------------
            for b in range(NBLK):
                feat = sb.tile([31, EPB], bf16, tag="feat")
                nc.scalar.dma_start(out=feat[:], in_=f31[b, :, :])
                ohE_b = sb.tile([128, EPB], bf16, tag="ohE1")
                nc.sync.dma_start(out=ohE_b[:], in_=ohTE_all[b, :, EPB:2 * EPB])
                FW = MAX_CPB * 12
                msg1 = ps_msg.tile([128, 2, 2 * H], f32, space="PSUM", tag="msgpair")
                for t in range(MAX_CPB):
                    nc.tensor.matmul(out=msg1[:, 0, t * 12:(t + 1) * 12],
                                     lhsT=feat[:, t * 128:(t + 1) * 128],
                                     rhs=W1f_t[:], start=True, stop=True)
                    nc.tensor.matmul(out=msg1[:, 1, t * 12:(t + 1) * 12],
                                     lhsT=feat[:, t * 128:(t + 1) * 128],
                                     rhs=W1s_t[:], start=True, stop=True)
                u1 = sb.tile([128, 2, FW], bf16, tag="u1")
                nc.scalar.activation(u1[:], msg1[:, :, 0:FW], AF.Exp)
                v1 = sb.tile([128, 2, FW], bf16, tag="v1")
                nc.scalar.activation(v1[:], u1[:], AF.Ln, bias=1.0)
                sig1 = sb.tile([128, FW], bf16, tag="sig1")
                nc.scalar.activation(sig1[:], v1[:, 0, :], AF.Exp, scale=-1.0)
                m1 = sb.tile([128, FW], bf16, tag="m1")
                nc.vector.tensor_tensor(out=m1[:], in0=sig1[:], in1=v1[:, 1, :], op=ALU.mult)
                scat = ps_scat.tile([128, 128], f32, space="PSUM", tag="scat")
                for t in range(MAX_CPB):
                    nc.tensor.matmul(out=scat[:F_NODE, :], lhsT=m1[:, t * 12:(t + 1) * 12],
                                     rhs=ohE_b[:, t * 128:(t + 1) * 128],
                                     start=(t == 0), stop=(t == MAX_CPB - 1))
                hadd = sb.tile([F_NODE, 128], f32, tag="hadd")
                nc.vector.tensor_tensor(out=hadd[:], in0=scat[:F_NODE, :],
                                        in1=xT12_t[:, b * 128:(b + 1) * 128], op=ALU.add)
                h1T = sb.tile([F_NODE, 128], bf16, tag="h1T")
                nc.scalar.activation(h1T[:], hadd[:], AF.Relu)
                hps_t = ps_scat.tile([128, 128], f32, space="PSUM", tag="scat")
                hps = hps_t[:]
                nc.tensor.matmul(out=hps, lhsT=Wlin12_t[:], rhs=h1T[:],
                                 start=True, stop=False)
                nc.tensor.matmul(out=hps, lhsT=WlinB_t[:], rhs=ones_row[:],
                                 start=False, stop=True)
                nc.vector.tensor_copy(out=hT[:, b * 128:(b + 1) * 128], in_=hps)
                hT_bf = sb.tile([H, 128], bf16, tag="hTbf")
                nc.vector.tensor_copy(out=hT_bf[:], in_=hps)
                dense_next(b, hT_bf, 2)
                if b in SEG_AT:
                    seg_ag(2, SEG_AT[b])

            # ---------------- LAYERS 2, 3 ----------------
            pool_acc = ps_pool.tile([128, H + 1], f32, space="PSUM", tag="pool")
            for L in (2, 3):
                for b in range(NBLK):
                    ohTE_b = sb.tile([128, 2 * EPB], bf16, tag="ohTEb")
                    nc.sync.dma_start(out=ohTE_b[:], in_=ohTE_all[b, :, :])
                    eTb = sb.tile([7, EPB], bf16, tag="eTb")
                    nc.scalar.dma_start(out=eTb[:], in_=eT7[:, b * EPB:(b + 1) * EPB])
                    scat = ps_scat.tile([128, 128], f32, space="PSUM", tag="scat")
                    for pi, pair in enumerate(PAIRS):
                        gp = gth.tile([128, 2, 2 * H], bf16, tag="gp")
                        for j, t in enumerate(pair):
                            tt = b * MAX_CPB + t
                            nc.gpsimd.indirect_dma_start(
                                out=gp[:, j, :], out_offset=None, in_=yfs_full[L][:],
                                in_offset=bass.IndirectOffsetOnAxis(ap=srcS_t[:, tt:tt + 1], axis=0))
                        msg = ps_msg.tile([128, 2, 2 * H], f32, space="PSUM", tag="msgpair")
                        for j, t in enumerate(pair):
                            nc.tensor.matmul(out=msg[:, j, :],
                                             lhsT=eTb[:, t * 128:(t + 1) * 128],
                                             rhs=We_t[L][:], start=True, stop=False)
                            nc.tensor.matmul(out=msg[:, j, :],
                                             lhsT=ohTE_b[:, t * 128:(t + 1) * 128],
                                             rhs=afs_all[:, b, :], start=False, stop=True)
                        npair = len(pair)
                        u0 = sb.tile([128, 2, 2 * H], bf16, tag="u0")
                        nc.scalar.activation(u0[:, :npair, :], msg[:, :npair, :], AF.Exp)
                        u = sb.tile([128, 2, 2 * H], bf16, tag="u")
                        nc.vector.tensor_tensor(out=u[:, :npair, :], in0=u0[:, :npair, :],
                                                in1=gp[:, :npair, :], op=ALU.mult)
                        v = sb.tile([128, 2, 2 * H], bf16, tag="v")
                        nc.scalar.activation(v[:, :npair, :], u[:, :npair, :], AF.Ln, bias=1.0)
                        sig = sb.tile([128, 2, H], bf16, tag="sig")
                        nc.scalar.activation(sig[:, :npair, :], v[:, :npair, 0:H],
                                             AF.Exp, scale=-1.0)
                        m = sb.tile([128, 2, H], bf16, tag="m")
                        nc.vector.tensor_tensor(out=m[:, :npair, :], in0=sig[:, :npair, :],
                                                in1=v[:, :npair, H:2 * H], op=ALU.mult)
                        for j, t in enumerate(pair):
                            nc.tensor.matmul(out=scat[:], lhsT=m[:, j, :],
                                             rhs=ohTE_b[:, EPB + t * 128:EPB + (t + 1) * 128],
                                             start=(t == 0), stop=(t == MAX_CPB - 1))
                    htmp = sb.tile([H, 128], f32, tag="htmp")
                    nc.vector.tensor_tensor(out=htmp[:], in0=scat[:],
                                            in1=hT[:, b * 128:(b + 1) * 128], op=ALU.add)
                    nc.vector.tensor_scalar_max(hT[:, b * 128:(b + 1) * 128], htmp[:], 0.0)
                    if L == 2:
                        hT_bf = sb.tile([H, 128], bf16, tag="hTbf")
                        nc.vector.tensor_copy(out=hT_bf[:], in_=hT[:, b * 128:(b + 1) * 128])
                        dense_next(b, hT_bf, 3)
                        if b in SEG_AT:
                            seg_ag(3, SEG_AT[b])
                    else:
                        # fused global_mean_pool accumulation
                        hT_bf = sb.tile([H, 128], bf16, tag="hTbf")
                        nc.vector.tensor_copy(out=hT_bf[:], in_=hT[:, b * 128:(b + 1) * 128])
                        tp = ps_tp.tile([128, 128], bf16, space="PSUM", tag="tp")
                        nc.tensor.transpose(out=tp[:], in_=hT_bf[:], identity=ident_bf[:])
                        haug = sb.tile([128, H + 1], bf16, tag="haug")
                        nc.vector.tensor_copy(out=haug[:, :H], in_=tp[:])
                        nc.vector.memset(haug[:, H:], 1.0)
                        ohB = oh.tile([128, 128], bf16, tag="ohB")
                        nc.vector.tensor_scalar(out=ohB[:], in0=iota_row_bf[:],
                                                scalar1=batchW_t[:, b:b + 1], scalar2=None,
                                                op0=ALU.is_equal)
                        nc.tensor.matmul(out=pool_acc[:], lhsT=ohB[:], rhs=haug[:],
                                         start=(b == 0), stop=(b == NBLK - 1))

            # ---------------- POOL ALLREDUCE + HEAD ----------------
            zt = sb.tile([128, H + 4], f32, tag="zt")
            nc.vector.memset(zt[:], 0.0)
            for r in range(8):
                nc.sync.dma_start(out=pool_in[r * 128:(r + 1) * 128, :], in_=zt[:])
            pool_sb = sb.tile([128, H + 4], f32, tag="poolsb")
            nc.vector.memset(pool_sb[:], 0.0)
            nc.vector.tensor_copy(out=pool_sb[:, :H + 1], in_=pool_acc[:])
            nc.gpsimd.indirect_dma_start(
                out=pool_in[:], out_offset=bass.IndirectOffsetOnAxis(ap=gidx_t[:, :1], axis=0),
                in_=pool_sb[:], in_offset=None)
            nc.gpsimd.collective_compute(
                "AllReduce", ALU.add, replica_groups=[list(range(C))],
                ins=[pool_in[:, :]], outs=[pool_out[:, :]])

            for r in range(8):
                pt = sb.tile([128, H + 4], f32, tag="pt")
                nc.sync.dma_start(out=pt[:], in_=pool_out[r * 128:(r + 1) * 128, :])
                cnt = sb.tile([128, 1], f32, tag="cnt")
                nc.vector.tensor_scalar_max(cnt[:], pt[:, H:H + 1], 1.0)
                rcnt = sb.tile([128, 1], f32, tag="rcnt")
                nc.vector.reciprocal(rcnt[:], cnt[:])
                gmean = sb.tile([128, H], bf16, tag="gmean")
                nc.vector.tensor_tensor(out=gmean[:], in0=pt[:, :H],
                                        in1=rcnt[:].to_broadcast([128, H]), op=ALU.mult)

                def dense_head(inp_bf, Wt, bt, n_out, relu):
                    tp = ps_tp.tile([128, 128], bf16, space="PSUM", tag="tp")
                    nc.tensor.transpose(out=tp[:], in_=inp_bf[:], identity=ident_bf[:])
                    tT = sb.tile([H, 128], bf16, tag="tT")
                    nc.vector.tensor_copy(out=tT[:], in_=tp[:])
                    ops = ps_dense.tile([128, 2 * H], f32, space="PSUM", tag="dense")
                    nc.tensor.matmul(out=ops[:, :n_out], lhsT=tT[:], rhs=Wt[:], start=True, stop=True)
                    of = sb.tile([128, n_out], f32, tag=f"of{n_out}")
                    nc.vector.tensor_tensor(out=of[:], in0=ops[:, :n_out], in1=bt[:], op=ALU.add)
                    if relu:
                        ob = sb.tile([128, n_out], bf16, tag=f"ob{n_out}")
                        nc.vector.tensor_scalar_max(ob[:], of[:], 0.0)
                        return ob
                    return of

                o1 = dense_head(gmean, Wh1_t, bh1_t, H, True)
                o2 = dense_head(o1, Wh2_t, bh2_t, H, True)
                o3 = dense_head(o2, Wh3_t, bh3_t, 4, False)
                nc.sync.dma_start(out=out[r * 128:(r + 1) * 128, :], in_=o3[:, 0:1])

    import concourse.bacc as _bacc
    _orig_tables = _bacc.get_activation_tables
    def _one_table(arch):
        t = _orig_tables(arch)
        keep = "natural_log_exp_and_others"
        out2 = {}
        for k, v in t.items():
            v = set(v)
            if k == keep:
                v = v | {AF.Exp, AF.Ln, AF.Relu, AF.Copy}
            else:
                v = {f for f in v if f not in (AF.Exp, AF.Ln, AF.Relu, AF.Copy)}
            out2[k] = v
        return out2
    _bacc.get_activation_tables = _one_table
    try:
        nc.compile()
    finally:
        _bacc.get_activation_tables = _orig_tables
    return nc


def run(nc, in_maps, trace=False):
    from concourse.bass_utils import run_bass_kernel_spmd
    res = run_bass_kernel_spmd(nc, in_maps, core_ids=list(range(C)), trace=trace)
    return res


# ---------------------------------------------------------------- entry point
_CACHE = {}

def kernel(**inputs):
    """Full-input CGConv GNN on 8 trn2 NeuronCores. Returns [1000, 1] float32."""
    meta, in_maps = prep(inputs)
    key = (meta["MAX_CPB"],)
    if key not in _CACHE:
        _CACHE[key] = build(meta)
    nc = _CACHE[key]
    res = run(nc, in_maps, trace=False)
    return np.asarray(res.results[0]["out"][:N_GRAPHS], dtype=np.float32)
